# revision 2
# baseline (speedup 1.0000x reference)
"""Block-sparse linear y = x @ W^T on 8 Trainium2 NeuronCores.

Strategy: the 32x32 block structure (50% block density, random scatter) is not
exploitable on a 128x128 PE array, so we densify W^T on the host (cheap: 8MB
of scatter-adds) and run a dense GEMM, sharded 4-way over tokens x 2-way over
out_features (8 cores, no collectives, minimal per-core HBM traffic).

Inputs are cast to float16 on the host: the PE runs fp16 at the same
1 MAC/cell/cycle as float32r (PE floor ~54.6us for the 2.15 GMAC/core), but
DMA traffic halves (20MB -> 12MB/core, ~34us at 358GB/s) so the kernel is
purely PE-bound, and LDWEIGHTS gets the 2x FWL fast path so the weight loads
hide entirely under the 512-cycle matmul streams. fp16 rel-err vs the fp32
reference is ~2.6e-4 (tolerance 2e-2). MM_DTYPE switches to bfloat16
(~2e-3) or float32r (exact-ish, DMA-bound) for A/B runs.

Schedule per core, two phases over token halves (psum banks = 4 m-tiles x
2 n-halves):
  Phase 1 (tokens 0:512): k-outer with x + W staged just-in-time (demand
  ~226GB/s < 358 supply, so the PE never waits after the first tile); both
  n-half W streams live simultaneously; the second-half x supertiles ride
  the queue's slack from mid-phase on. Banks evict to SBUF on vector (n=0)
  + scalar (n=1) engines as each bank's k=15 matmul retires.
  Phase 2 (tokens 512:1024): m-outer k-inner (everything is resident), so
  each bank drains the moment its 32-matmul chain finishes; y rows are
  written as single 512KB DMAs with 4KB contiguous runs. The last m-tile
  splits its drain across the vector+sync and scalar queues to shorten
  the tail.

No warmup matmuls: with the fp16 DMA head (~1.2us) they would delay real
work past data-ready; the HAM k=4->8 clock ramp overlaps the first real
matmuls instead.
"""

import numpy as np

TOKENS, IN_F, OUT_F = 4096, 2048, 2048
BLOCK = 32
N_CORES = 8
TG, OG = 4, 2  # token groups x out-feature groups
T_SH = TOKENS // TG  # 1024 tokens per core
O_SH = OUT_F // OG  # 1024 out features per core
P = 128
NFREE = 512  # PSUM bank free dim (fp32)
KT = IN_F // P  # 16 k tiles
XH = T_SH // 2  # 512-token halves (phase 1 / phase 2)
MH = XH // P  # 4 m-tiles per half

MM_DTYPE = "float16"  # "float16" | "bfloat16" | "float32r" | "float32"
TRACE = False  # set by test.py to capture an NTFF profile

_nc_cache = {}
_last_result = None  # BassKernelResults of the most recent run (for test.py)


def _build_nc():
    import concourse.mybir as mybir
    import concourse.tile as tile
    from concourse import bacc

    key = MM_DTYPE
    if key in _nc_cache:
        return _nc_cache[key]

    dt_mm = getattr(mybir.dt, MM_DTYPE)
    f32 = mybir.dt.float32

    nc = bacc.Bacc(None, target_bir_lowering=False)
    # Host-pre-blocked inputs (exact SBUF layouts; all DMAs are linear):
    # xa: x^T by token-half, [2][P][KT][XH]; wn: W^T by n-half, [2][P][KT][NFREE]
    xa = nc.dram_tensor("xa", [2, P, KT, XH], dt_mm, kind="ExternalInput")
    wn = nc.dram_tensor("wn", [2, P, KT, NFREE], dt_mm, kind="ExternalInput")
    y = nc.dram_tensor("y", [T_SH, O_SH], f32, kind="ExternalOutput")

    with tile.TileContext(nc) as tc:
        with (
            tc.tile_pool(name="xp", bufs=1) as xp,
            tc.tile_pool(name="wp", bufs=1) as wp,
            tc.tile_pool(name="op", bufs=8) as op,
            tc.tile_pool(name="ps", bufs=1, space="PSUM") as ps,
        ):
            # stationary x^T [P, 128] and moving W^T [P, NFREE] slice getters;
            # k=0,1 are single-k tiles (small DMAs for a fast head), the rest
            # adjacent-k pairs (2KB contiguous runs at line rate).
            xsl = [[None] * KT, [None] * KT]  # [half][k] -> lambda m: AP
            wsl = [[None] * KT, [None] * KT]  # [n][k] -> AP

            def load_x(h, k, paired):
                if paired:
                    t = xp.tile([P, 2, XH], dt_mm, tag=f"x{h}_{k}", name=f"x{h}_{k}")
                    nc.sync.dma_start(t[:], xa[h, :, k : k + 2, :])
                    for kk in (k, k + 1):
                        xsl[h][kk] = (
                            lambda m, t=t, kk=kk: t[:, kk - k, m * P : (m + 1) * P]
                        )
                else:
                    t = xp.tile([P, XH], dt_mm, tag=f"x{h}_{k}", name=f"x{h}_{k}")
                    nc.sync.dma_start(t[:], xa[h, :, k, :])
                    xsl[h][k] = lambda m, t=t: t[:, m * P : (m + 1) * P]

            def load_w(n, k, paired):
                if paired:
                    t = wp.tile([P, 2, NFREE], dt_mm, tag=f"w{n}_{k}", name=f"w{n}_{k}")
                    nc.sync.dma_start(t[:], wn[n, :, k : k + 2, :])
                    wsl[n][k] = t[:, 0, :]
                    wsl[n][k + 1] = t[:, 1, :]
                else:
                    t = wp.tile([P, NFREE], dt_mm, tag=f"w{n}_{k}", name=f"w{n}_{k}")
                    nc.sync.dma_start(t[:], wn[n, :, k, :])
                    wsl[n][k] = t[:]

            def fresh_psums():
                return [
                    ps.tile([P, NFREE], f32, tag=f"ps{b}", name=f"ps{b}")
                    for b in range(8)
                ]

            # ---- DMA stream (sync queue, consumption-priority order) ----
            for k in (0, 1):  # singles: first matmul waits on ~256KB only
                load_x(0, k, False)
                load_w(0, k, False)
                load_w(1, k, False)
            for j, k in enumerate(range(2, KT, 2)):  # pairs for k=2..15
                load_x(0, k, True)
                load_w(0, k, True)
                load_w(1, k, True)
                if j >= 1:  # slot phase-2 x pairs into the stream's slack
                    load_x(1, 2 * (j - 1), True)
            for k in range(2 * (KT // 2 - 2), KT, 2):  # remaining phase-2 x
                load_x(1, k, True)

            # ---- Phase 1: tokens 0:XH, k-outer, both n-halves per bank pair ----
            psums = fresh_psums()
            for k in range(KT):
                for m in range(MH):
                    st = xsl[0][k](m)
                    for n in range(2):
                        nc.tensor.matmul(
                            psums[2 * m + n],
                            st,
                            wsl[n][k],
                            start=(k == 0),
                            stop=(k == KT - 1),
                        )
            for m in range(MH):  # evict: vector takes n=0, scalar takes n=1
                yt = op.tile([P, 2 * NFREE], f32, tag="ot")
                nc.vector.tensor_copy(yt[:, :NFREE], psums[2 * m][:])
                nc.scalar.copy(yt[:, NFREE:], psums[2 * m + 1][:])
                nc.scalar.dma_start(y[m * P : (m + 1) * P, :], yt[:])

            # ---- Phase 2: tokens XH:2*XH, m-outer k-inner, per-bank drain ----
            psums = fresh_psums()
            for m in range(MH):
                b0, b1 = 2 * m, 2 * m + 1
                for k in range(KT):
                    st = xsl[1][k](m)
                    for n in range(2):
                        nc.tensor.matmul(
                            psums[b0 + n],
                            st,
                            wsl[n][k],
                            start=(k == 0),
                            stop=(k == KT - 1),
                        )
                r = slice((MH + m) * P, (MH + m + 1) * P)
                if m < MH - 1:
                    yt = op.tile([P, 2 * NFREE], f32, tag="ot")
                    nc.vector.tensor_copy(yt[:, :NFREE], psums[b0][:])
                    nc.scalar.copy(yt[:, NFREE:], psums[b1][:])
                    nc.scalar.dma_start(y[r, :], yt[:])
                else:  # last tile: split the drain across engines + queues
                    ot0 = op.tile([P, NFREE], f32, tag="ot")
                    nc.vector.tensor_copy(ot0[:], psums[b0][:])
                    nc.sync.dma_start(y[r, :NFREE], ot0[:])
                    ot1 = op.tile([P, NFREE], f32, tag="ot")
                    nc.scalar.copy(ot1[:], psums[b1][:])
                    nc.scalar.dma_start(y[r, NFREE:], ot1[:])

    nc.compile()
    _nc_cache[key] = nc
    return nc


def _densify_wT(weight_blocks, block_rows, block_cols):
    """Scatter-add the 32x32 blocks into dense W^T [in_features, out_features]."""
    nc_blk = IN_F // BLOCK
    nr_blk = OUT_F // BLOCK
    wcr = np.zeros((nc_blk, nr_blk, BLOCK, BLOCK), np.float32)
    # block b occupies W[32r:32r+32, 32c:32c+32]; W^T gets the transposed block
    np.add.at(
        wcr,
        (block_cols.astype(np.int64), block_rows.astype(np.int64)),
        np.swapaxes(weight_blocks.astype(np.float32, copy=False), 1, 2),
    )
    return np.ascontiguousarray(wcr.transpose(0, 2, 1, 3).reshape(IN_F, OUT_F))


def _pack_core_inputs(xT_sh, wT_sh, np_dt):
    """Block one core's x^T and W^T shards into the kernel's DMA layouts."""
    xk = xT_sh.reshape(KT, P, 2, XH)  # [k, p, token-half, t]
    wk = wT_sh.reshape(KT, P, 2, NFREE)  # [k, p, n-half, f]
    return {
        "xa": xk.transpose(2, 1, 0, 3).astype(np_dt),  # [2, P, KT, XH]
        "wn": wk.transpose(2, 1, 0, 3).astype(np_dt),  # [2, P, KT, NFREE]
    }


def kernel(x, weight_blocks, block_rows, block_cols):
    global _last_result
    from concourse.bass_utils import run_bass_kernel_spmd

    if MM_DTYPE == "float16":
        np_dt = np.float16
    elif MM_DTYPE == "bfloat16":
        import ml_dtypes

        np_dt = ml_dtypes.bfloat16
    else:
        np_dt = np.float32

    x = np.asarray(x, dtype=np.float32)
    wT = _densify_wT(
        np.asarray(weight_blocks), np.asarray(block_rows), np.asarray(block_cols)
    )
    xT = np.ascontiguousarray(x.T)

    in_maps = []
    for c in range(N_CORES):
        tg, og = divmod(c, OG)
        in_maps.append(
            _pack_core_inputs(
                xT[:, tg * T_SH : (tg + 1) * T_SH],
                wT[:, og * O_SH : (og + 1) * O_SH],
                np_dt,
            )
        )

    nc = _build_nc()
    res = None
    for attempt in range(3):  # transient NRT device errors happen; retry
        try:
            res = run_bass_kernel_spmd(
                nc, in_maps, core_ids=list(range(N_CORES)), trace=TRACE
            )
            break
        except Exception:
            if attempt == 2:
                raise
            import time

            time.sleep(3)
    _last_result = res

    y = np.empty((TOKENS, OUT_F), np.float32)
    for c in range(N_CORES):
        tg, og = divmod(c, OG)
        y[tg * T_SH : (tg + 1) * T_SH, og * O_SH : (og + 1) * O_SH] = res.results[c][
            "y"
        ]
    return y


# revision 6
# speedup vs baseline: 1.1583x; 1.1583x over previous
"""Block-sparse linear y = x @ W^T on 8 Trainium2 NeuronCores.

Strategy: the 32x32 block structure (50% block density, random scatter) is not
exploitable on a 128x128 PE array, so we densify W^T on the host (cheap: 8MB
of scatter-adds) and run a dense GEMM, sharded 4-way over tokens x 2-way over
out_features (8 cores, no collectives, minimal per-core HBM traffic).

Inputs are cast to float16 on the host: the PE runs fp16 at the same
1 MAC/cell/cycle as float32r (PE floor ~54.6us for the 2.15 GMAC/core), but
DMA traffic halves (20MB -> 12MB/core, ~34us at 358GB/s) so the kernel is
purely PE-bound, and LDWEIGHTS gets the 2x FWL fast path so the weight loads
hide entirely under the 512-cycle matmul streams. fp16 rel-err vs the fp32
reference is ~2.6e-4 (tolerance 2e-2). MM_DTYPE switches to bfloat16
(~2e-3) or float32r (exact-ish, DMA-bound) for A/B runs.

Schedule per core, two phases over token halves (psum banks = 4 m-tiles x
2 n-halves):
  Phase 1 (tokens 0:512): k-outer with x + W staged just-in-time (demand
  ~226GB/s < 358 supply, so the PE never waits after the first tile); both
  n-half W streams live simultaneously; the second-half x supertiles ride
  the queue's slack from mid-phase on. Banks evict to SBUF on vector (n=0)
  + scalar (n=1) engines as each bank's k=15 matmul retires.
  Phase 2 (tokens 512:1024): m-outer k-inner (everything is resident), so
  each bank drains the moment its 32-matmul chain finishes; y rows are
  written as single 512KB DMAs with 4KB contiguous runs. The last m-tile
  splits its drain across the vector+sync and scalar queues to shorten
  the tail.

No warmup matmuls: with the fp16 DMA head (~1.2us) they would delay real
work past data-ready; the HAM k=4->8 clock ramp overlaps the first real
matmuls instead.
"""

import numpy as np

TOKENS, IN_F, OUT_F = 4096, 2048, 2048
BLOCK = 32
N_CORES = 8
TG, OG = 4, 2  # token groups x out-feature groups
T_SH = TOKENS // TG  # 1024 tokens per core
O_SH = OUT_F // OG  # 1024 out features per core
P = 128
NFREE = 512  # PSUM bank free dim (fp32)
KT = IN_F // P  # 16 k tiles
XH = T_SH // 2  # 512-token halves (phase 1 / phase 2)
MH = XH // P  # 4 m-tiles per half

MM_DTYPE = "bfloat16"  # "bfloat16" | "float16" | "float32r" | "float32"
WARM_MMS = 5  # dummy matmuls to lift the HAM clock gate during the DMA head
TRACE = False  # set by test.py to capture an NTFF profile

_nc_cache = {}
_last_result = None  # BassKernelResults of the most recent run (for test.py)


def _build_nc():
    import concourse.mybir as mybir
    import concourse.tile as tile
    from concourse import bacc

    key = MM_DTYPE
    if key in _nc_cache:
        return _nc_cache[key]

    dt_mm = getattr(mybir.dt, MM_DTYPE)
    f32 = mybir.dt.float32

    nc = bacc.Bacc(None, target_bir_lowering=False)
    # Host-pre-blocked inputs (exact SBUF layouts; all DMAs are linear):
    # xa: x^T by token-half, [2][P][KT][XH]; wn: W^T by n-half, [2][P][KT][NFREE]
    xa = nc.dram_tensor("xa", [2, P, KT, XH], dt_mm, kind="ExternalInput")
    wn = nc.dram_tensor("wn", [2, P, KT, NFREE], dt_mm, kind="ExternalInput")
    y = nc.dram_tensor("y", [T_SH, O_SH], f32, kind="ExternalOutput")

    with tile.TileContext(nc) as tc:
        with (
            tc.tile_pool(name="xp", bufs=1) as xp,
            tc.tile_pool(name="wp", bufs=1) as wp,
            tc.tile_pool(name="op", bufs=8) as op,
            tc.tile_pool(name="ps", bufs=1, space="PSUM") as ps,
        ):
            # Warm the PE's HAM clock gate during the DMA head (~3us idle
            # before the first tiles land): dummy matmuls on a zeroed tile
            # take the array past the 3.4us busy window so the first real
            # matmuls run at 2.4GHz instead of 1.2.
            if WARM_MMS:
                zt = xp.tile([P, NFREE], dt_mm, tag="warm", name="warm")
                nc.gpsimd.memset(zt[:], 0.0)
                warm_ps = ps.tile([P, NFREE], f32, tag="ps0", name="warm_ps")
                for _ in range(WARM_MMS):
                    nc.tensor.matmul(warm_ps[:], zt[:, :P], zt[:], start=True, stop=True)

            # stationary x^T [P, 128] and moving W^T [P, NFREE] slice getters;
            # k=0,1 are single-k tiles (small DMAs for a fast head), the rest
            # adjacent-k pairs (2KB contiguous runs at line rate).
            xsl = [[None] * KT, [None] * KT]  # [half][k] -> lambda m: AP
            wsl = [[None] * KT, [None] * KT]  # [n][k] -> AP

            def load_x(h, k, paired):
                if paired:
                    t = xp.tile([P, 2, XH], dt_mm, tag=f"x{h}_{k}", name=f"x{h}_{k}")
                    nc.sync.dma_start(t[:], xa[h, :, k : k + 2, :])
                    for kk in (k, k + 1):
                        xsl[h][kk] = (
                            lambda m, t=t, kk=kk: t[:, kk - k, m * P : (m + 1) * P]
                        )
                else:
                    t = xp.tile([P, XH], dt_mm, tag=f"x{h}_{k}", name=f"x{h}_{k}")
                    nc.sync.dma_start(t[:], xa[h, :, k, :])
                    xsl[h][k] = lambda m, t=t: t[:, m * P : (m + 1) * P]

            def load_w(n, k, paired, eng=None):
                if paired:
                    t = wp.tile([P, 2, NFREE], dt_mm, tag=f"w{n}_{k}", name=f"w{n}_{k}")
                    nc.sync.dma_start(t[:], wn[n, :, k : k + 2, :])
                    wsl[n][k] = t[:, 0, :]
                    wsl[n][k + 1] = t[:, 1, :]
                else:
                    t = wp.tile([P, NFREE], dt_mm, tag=f"w{n}_{k}", name=f"w{n}_{k}")
                    (eng or nc.sync).dma_start(t[:], wn[n, :, k, :])
                    wsl[n][k] = t[:]

            def fresh_psums():
                return [
                    ps.tile([P, NFREE], f32, tag=f"ps{b}", name=f"ps{b}")
                    for b in range(8)
                ]

            # ---- DMA stream (sync queue, consumption-priority order).
            # The very first w tile goes on the (otherwise idle) gpsimd
            # SWDGE queue so xah0 + w00 transfer in parallel. ----
            for k in (0, 1):  # singles: first matmul waits on ~256KB only
                load_x(0, k, False)
                load_w(0, k, False, eng=nc.gpsimd if k == 0 else None)
                load_w(1, k, False)
            for j, k in enumerate(range(2, KT, 2)):  # pairs for k=2..15
                load_x(0, k, True)
                load_w(0, k, True)
                load_w(1, k, True)
                if j >= 1:  # slot phase-2 x pairs into the stream's slack
                    load_x(1, 2 * (j - 1), True)
            for k in range(2 * (KT // 2 - 2), KT, 2):  # remaining phase-2 x
                load_x(1, k, True)

            # ---- Phase 1: tokens 0:XH, k-outer, both n-halves per bank pair ----
            psums = fresh_psums()
            for k in range(KT):
                for m in range(MH):
                    st = xsl[0][k](m)
                    for n in range(2):
                        nc.tensor.matmul(
                            psums[2 * m + n],
                            st,
                            wsl[n][k],
                            start=(k == 0),
                            stop=(k == KT - 1),
                        )
            for m in range(MH):  # evict: vector takes n=0, scalar takes n=1
                yt = op.tile([P, 2 * NFREE], f32, tag="ot")
                nc.vector.tensor_copy(yt[:, :NFREE], psums[2 * m][:])
                nc.scalar.copy(yt[:, NFREE:], psums[2 * m + 1][:])
                nc.scalar.dma_start(y[m * P : (m + 1) * P, :], yt[:])

            # ---- Phase 2: tokens XH:2*XH, m-outer k-inner, per-bank drain ----
            psums = fresh_psums()
            for m in range(MH):
                b0, b1 = 2 * m, 2 * m + 1
                for k in range(KT):
                    st = xsl[1][k](m)
                    for n in range(2):
                        nc.tensor.matmul(
                            psums[b0 + n],
                            st,
                            wsl[n][k],
                            start=(k == 0),
                            stop=(k == KT - 1),
                        )
                r = slice((MH + m) * P, (MH + m + 1) * P)
                if m < MH - 1:
                    yt = op.tile([P, 2 * NFREE], f32, tag="ot")
                    nc.vector.tensor_copy(yt[:, :NFREE], psums[b0][:])
                    nc.scalar.copy(yt[:, NFREE:], psums[b1][:])
                    nc.scalar.dma_start(y[r, :], yt[:])
                else:  # last tile: split the drain across engines + queues
                    ot0 = op.tile([P, NFREE], f32, tag="ot")
                    nc.vector.tensor_copy(ot0[:], psums[b0][:])
                    nc.sync.dma_start(y[r, :NFREE], ot0[:])
                    ot1 = op.tile([P, NFREE], f32, tag="ot")
                    nc.scalar.copy(ot1[:], psums[b1][:])
                    nc.scalar.dma_start(y[r, NFREE:], ot1[:])

    nc.compile()
    _nc_cache[key] = nc
    return nc


def _densify_wT(weight_blocks, block_rows, block_cols):
    """Scatter-add the 32x32 blocks into dense W^T [in_features, out_features]."""
    nc_blk = IN_F // BLOCK
    nr_blk = OUT_F // BLOCK
    wcr = np.zeros((nc_blk, nr_blk, BLOCK, BLOCK), np.float32)
    # block b occupies W[32r:32r+32, 32c:32c+32]; W^T gets the transposed block
    np.add.at(
        wcr,
        (block_cols.astype(np.int64), block_rows.astype(np.int64)),
        np.swapaxes(weight_blocks.astype(np.float32, copy=False), 1, 2),
    )
    return np.ascontiguousarray(wcr.transpose(0, 2, 1, 3).reshape(IN_F, OUT_F))


def _pack_core_inputs(xT_sh, wT_sh, np_dt):
    """Block one core's x^T and W^T shards into the kernel's DMA layouts."""
    xk = xT_sh.reshape(KT, P, 2, XH)  # [k, p, token-half, t]
    wk = wT_sh.reshape(KT, P, 2, NFREE)  # [k, p, n-half, f]
    return {
        "xa": xk.transpose(2, 1, 0, 3).astype(np_dt),  # [2, P, KT, XH]
        "wn": wk.transpose(2, 1, 0, 3).astype(np_dt),  # [2, P, KT, NFREE]
    }


def kernel(x, weight_blocks, block_rows, block_cols):
    global _last_result
    from concourse.bass_utils import run_bass_kernel_spmd

    if MM_DTYPE == "float16":
        np_dt = np.float16
    elif MM_DTYPE == "bfloat16":
        import ml_dtypes

        np_dt = ml_dtypes.bfloat16
    else:
        np_dt = np.float32

    x = np.asarray(x, dtype=np.float32)
    wT = _densify_wT(
        np.asarray(weight_blocks), np.asarray(block_rows), np.asarray(block_cols)
    )
    xT = np.ascontiguousarray(x.T)

    in_maps = []
    for c in range(N_CORES):
        tg, og = divmod(c, OG)
        in_maps.append(
            _pack_core_inputs(
                xT[:, tg * T_SH : (tg + 1) * T_SH],
                wT[:, og * O_SH : (og + 1) * O_SH],
                np_dt,
            )
        )

    nc = _build_nc()
    res = None
    for attempt in range(3):  # transient NRT device errors happen; retry
        try:
            res = run_bass_kernel_spmd(
                nc, in_maps, core_ids=list(range(N_CORES)), trace=TRACE
            )
            break
        except Exception:
            if attempt == 2:
                raise
            import time

            time.sleep(3)
    _last_result = res

    y = np.empty((TOKENS, OUT_F), np.float32)
    for c in range(N_CORES):
        tg, og = divmod(c, OG)
        y[tg * T_SH : (tg + 1) * T_SH, og * O_SH : (og + 1) * O_SH] = res.results[c][
            "y"
        ]
    return y


# revision 13
# speedup vs baseline: 1.1603x; 1.0017x over previous
"""Block-sparse linear y = x @ W^T on 8 Trainium2 NeuronCores.

Strategy: the 32x32 block structure (50% block density, random scatter) is not
exploitable on a 128x128 PE array, so we densify W^T on the host (cheap: 8MB
of scatter-adds) and run a dense GEMM, sharded 4-way over tokens x 2-way over
out_features (8 cores, no collectives, minimal per-core HBM traffic).

Inputs are cast to float16 on the host: the PE runs fp16 at the same
1 MAC/cell/cycle as float32r (PE floor ~54.6us for the 2.15 GMAC/core), but
DMA traffic halves (20MB -> 12MB/core, ~34us at 358GB/s) so the kernel is
purely PE-bound, and LDWEIGHTS gets the 2x FWL fast path so the weight loads
hide entirely under the 512-cycle matmul streams. fp16 rel-err vs the fp32
reference is ~2.6e-4 (tolerance 2e-2). MM_DTYPE switches to bfloat16
(~2e-3) or float32r (exact-ish, DMA-bound) for A/B runs.

Schedule per core, two phases over token halves (psum banks = 4 m-tiles x
2 n-halves):
  Phase 1 (tokens 0:512): k-outer with x + W staged just-in-time (demand
  ~226GB/s < 358 supply, so the PE never waits after the first tile); both
  n-half W streams live simultaneously; the second-half x supertiles ride
  the queue's slack from mid-phase on. Banks evict to SBUF on vector (n=0)
  + scalar (n=1) engines as each bank's k=15 matmul retires.
  Phase 2 (tokens 512:1024): m-outer k-inner (everything is resident), so
  each bank drains the moment its 32-matmul chain finishes; y rows are
  written as single 512KB DMAs with 4KB contiguous runs. The last m-tile
  splits its drain across the vector+sync and scalar queues to shorten
  the tail.

No warmup matmuls: with the fp16 DMA head (~1.2us) they would delay real
work past data-ready; the HAM k=4->8 clock ramp overlaps the first real
matmuls instead.
"""

import numpy as np

TOKENS, IN_F, OUT_F = 4096, 2048, 2048
BLOCK = 32
N_CORES = 8
TG, OG = 4, 2  # token groups x out-feature groups
T_SH = TOKENS // TG  # 1024 tokens per core
O_SH = OUT_F // OG  # 1024 out features per core
P = 128
NFREE = 512  # PSUM bank free dim (fp32)
KT = IN_F // P  # 16 k tiles
XH = T_SH // 2  # 512-token halves (phase 1 / phase 2)
MH = XH // P  # 4 m-tiles per half

MM_DTYPE = "bfloat16"  # "bfloat16" | "float16" | "float32r" | "float32"
WARM_MMS = 9  # dummy matmuls to lift the HAM clock gate during the DMA head
Y_BF16 = True  # write y as bf16 (halves drain traffic), host upcasts to fp32
TRACE = False  # set by test.py to capture an NTFF profile

_nc_cache = {}
_last_result = None  # BassKernelResults of the most recent run (for test.py)


def _build_nc():
    import concourse.mybir as mybir
    import concourse.tile as tile
    from concourse import bacc

    key = MM_DTYPE
    if key in _nc_cache:
        return _nc_cache[key]

    dt_mm = getattr(mybir.dt, MM_DTYPE)
    f32 = mybir.dt.float32
    dt_y = mybir.dt.bfloat16 if Y_BF16 else f32

    nc = bacc.Bacc(None, target_bir_lowering=False)
    # Host-pre-blocked inputs (exact SBUF layouts; all DMAs are linear):
    # xa: x^T by token-half, [2][P][KT][XH]; wn: W^T by n-half, [2][P][KT][NFREE]
    xa = nc.dram_tensor("xa", [2, P, KT, XH], dt_mm, kind="ExternalInput")
    wn = nc.dram_tensor("wn", [2, P, KT, NFREE], dt_mm, kind="ExternalInput")
    y = nc.dram_tensor("y", [T_SH, O_SH], dt_y, kind="ExternalOutput")

    with tile.TileContext(nc) as tc:
        with (
            tc.tile_pool(name="xp", bufs=1) as xp,
            tc.tile_pool(name="wp", bufs=1) as wp,
            tc.tile_pool(name="op", bufs=8) as op,
            tc.tile_pool(name="ps", bufs=1, space="PSUM") as ps,
        ):
            # Warm the PE's HAM clock gate during the DMA head (~3us idle
            # before the first tiles land): dummy matmuls on a zeroed tile
            # take the array past the 3.4us busy window so the first real
            # matmuls run at 2.4GHz instead of 1.2.
            if WARM_MMS:
                zt = xp.tile([P, NFREE], dt_mm, tag="warm", name="warm")
                nc.gpsimd.memset(zt[:], 0.0)
                warm_ps = ps.tile([P, NFREE], f32, tag="ps0", name="warm_ps")
                for _ in range(WARM_MMS):
                    nc.tensor.matmul(warm_ps[:], zt[:, :P], zt[:], start=True, stop=True)

            # stationary x^T [P, 128] and moving W^T [P, NFREE] slice getters;
            # k=0,1 are single-k tiles (small DMAs for a fast head), the rest
            # adjacent-k pairs (2KB contiguous runs at line rate).
            xsl = [[None] * KT, [None] * KT]  # [half][k] -> lambda m: AP
            wsl = [[None] * KT, [None] * KT]  # [n][k] -> AP

            def load_x(h, k, paired):
                if paired:
                    t = xp.tile([P, 2, XH], dt_mm, tag=f"x{h}_{k}", name=f"x{h}_{k}")
                    nc.sync.dma_start(t[:], xa[h, :, k : k + 2, :])
                    for kk in (k, k + 1):
                        xsl[h][kk] = (
                            lambda m, t=t, kk=kk: t[:, kk - k, m * P : (m + 1) * P]
                        )
                else:
                    t = xp.tile([P, XH], dt_mm, tag=f"x{h}_{k}", name=f"x{h}_{k}")
                    nc.sync.dma_start(t[:], xa[h, :, k, :])
                    xsl[h][k] = lambda m, t=t: t[:, m * P : (m + 1) * P]

            def load_w(n, k, paired, eng=None):
                if paired:
                    t = wp.tile([P, 2, NFREE], dt_mm, tag=f"w{n}_{k}", name=f"w{n}_{k}")
                    (eng or nc.sync).dma_start(t[:], wn[n, :, k : k + 2, :])
                    wsl[n][k] = t[:, 0, :]
                    wsl[n][k + 1] = t[:, 1, :]
                else:
                    t = wp.tile([P, NFREE], dt_mm, tag=f"w{n}_{k}", name=f"w{n}_{k}")
                    (eng or nc.sync).dma_start(t[:], wn[n, :, k, :])
                    wsl[n][k] = t[:]

            def fresh_psums():
                return [
                    ps.tile([P, NFREE], f32, tag=f"ps{b}", name=f"ps{b}")
                    for b in range(8)
                ]

            # ---- DMA stream (sync queue, consumption-priority order).
            # All loads are adjacent-k pairs: per-DMA issue cost (~0.7us of
            # engine time) makes single-k loads supply-rate-limited. The
            # first w0 pair rides the (otherwise idle) gpsimd SWDGE queue so
            # it transfers in parallel with the first x pair. ----
            for j, k in enumerate(range(0, KT, 2)):
                load_x(0, k, True)
                load_w(0, k, True, eng=nc.gpsimd if k == 0 else None)
                load_w(1, k, True)
                if j >= 2:  # slot phase-2 x pairs into the stream's slack
                    load_x(1, 2 * (j - 2), True)
            for k in range(2 * (KT // 2 - 2), KT, 2):  # remaining phase-2 x
                load_x(1, k, True)

            # ---- Phase 1: tokens 0:XH, k-outer, both n-halves per bank pair ----
            psums = fresh_psums()
            for k in range(KT):
                for m in range(MH):
                    st = xsl[0][k](m)
                    for n in range(2):
                        nc.tensor.matmul(
                            psums[2 * m + n],
                            st,
                            wsl[n][k],
                            start=(k == 0),
                            stop=(k == KT - 1),
                        )
            for m in range(MH):  # evict: vector takes n=0, scalar takes n=1
                yt = op.tile([P, 2 * NFREE], dt_y, tag="ot")
                nc.vector.tensor_copy(yt[:, :NFREE], psums[2 * m][:])
                nc.scalar.copy(yt[:, NFREE:], psums[2 * m + 1][:])
                nc.scalar.dma_start(y[m * P : (m + 1) * P, :], yt[:])

            # ---- Phase 2: tokens XH:2*XH, m-outer k-inner, per-bank drain ----
            psums = fresh_psums()
            for m in range(MH):
                b0, b1 = 2 * m, 2 * m + 1
                for k in range(KT):
                    st = xsl[1][k](m)
                    for n in range(2):
                        nc.tensor.matmul(
                            psums[b0 + n],
                            st,
                            wsl[n][k],
                            start=(k == 0),
                            stop=(k == KT - 1),
                        )
                r = slice((MH + m) * P, (MH + m + 1) * P)
                if m < MH - 1:
                    yt = op.tile([P, 2 * NFREE], dt_y, tag="ot")
                    nc.vector.tensor_copy(yt[:, :NFREE], psums[b0][:])
                    nc.scalar.copy(yt[:, NFREE:], psums[b1][:])
                    nc.scalar.dma_start(y[r, :], yt[:])
                else:  # last tile: split the drain across engines + queues
                    ot0 = op.tile([P, NFREE], dt_y, tag="ot")
                    nc.vector.tensor_copy(ot0[:], psums[b0][:])
                    nc.sync.dma_start(y[r, :NFREE], ot0[:])
                    ot1 = op.tile([P, NFREE], dt_y, tag="ot")
                    nc.scalar.copy(ot1[:], psums[b1][:])
                    nc.scalar.dma_start(y[r, NFREE:], ot1[:])

    nc.compile()
    _nc_cache[key] = nc
    return nc


def _densify_wT(weight_blocks, block_rows, block_cols):
    """Scatter-add the 32x32 blocks into dense W^T [in_features, out_features]."""
    nc_blk = IN_F // BLOCK
    nr_blk = OUT_F // BLOCK
    wcr = np.zeros((nc_blk, nr_blk, BLOCK, BLOCK), np.float32)
    # block b occupies W[32r:32r+32, 32c:32c+32]; W^T gets the transposed block
    np.add.at(
        wcr,
        (block_cols.astype(np.int64), block_rows.astype(np.int64)),
        np.swapaxes(weight_blocks.astype(np.float32, copy=False), 1, 2),
    )
    return np.ascontiguousarray(wcr.transpose(0, 2, 1, 3).reshape(IN_F, OUT_F))


def _pack_core_inputs(xT_sh, wT_sh, np_dt):
    """Block one core's x^T and W^T shards into the kernel's DMA layouts."""
    xk = xT_sh.reshape(KT, P, 2, XH)  # [k, p, token-half, t]
    wk = wT_sh.reshape(KT, P, 2, NFREE)  # [k, p, n-half, f]
    return {
        "xa": xk.transpose(2, 1, 0, 3).astype(np_dt),  # [2, P, KT, XH]
        "wn": wk.transpose(2, 1, 0, 3).astype(np_dt),  # [2, P, KT, NFREE]
    }


def kernel(x, weight_blocks, block_rows, block_cols):
    global _last_result
    from concourse.bass_utils import run_bass_kernel_spmd

    if MM_DTYPE == "float16":
        np_dt = np.float16
    elif MM_DTYPE == "bfloat16":
        import ml_dtypes

        np_dt = ml_dtypes.bfloat16
    else:
        np_dt = np.float32

    x = np.asarray(x, dtype=np.float32)
    wT = _densify_wT(
        np.asarray(weight_blocks), np.asarray(block_rows), np.asarray(block_cols)
    )
    xT = np.ascontiguousarray(x.T)

    in_maps = []
    for c in range(N_CORES):
        tg, og = divmod(c, OG)
        in_maps.append(
            _pack_core_inputs(
                xT[:, tg * T_SH : (tg + 1) * T_SH],
                wT[:, og * O_SH : (og + 1) * O_SH],
                np_dt,
            )
        )

    nc = _build_nc()
    res = None
    for attempt in range(3):  # transient NRT device errors happen; retry
        try:
            res = run_bass_kernel_spmd(
                nc, in_maps, core_ids=list(range(N_CORES)), trace=TRACE
            )
            break
        except Exception:
            if attempt == 2:
                raise
            import time

            time.sleep(3)
    _last_result = res

    y = np.empty((TOKENS, OUT_F), np.float32)
    for c in range(N_CORES):
        tg, og = divmod(c, OG)
        y[tg * T_SH : (tg + 1) * T_SH, og * O_SH : (og + 1) * O_SH] = np.asarray(
            res.results[c]["y"]
        ).astype(np.float32)
    return y
